# revision 30
# baseline (speedup 1.0000x reference)
"""Trainium2 Bass kernel: 2-layer GraphSAGE (mean aggregation), 8-core SPMD.

nn_BiGNN: out = sage2(relu(sage1(x)));  sage(x) = mean_{j->i}(x_j) @ W_l + b_l + x @ W_r
N=50000 nodes, E=800000 edges, d=128, f32 inputs / f32 output.

Strategy (one NeuronCore owns 6250 destination nodes):
  - host: partition edges by destination block, sort by dst, pad per
    128-dst subwindow, split into lo/hi phases (int16 SWDGE index limit),
    equalize batch counts across cores (SPMD). Separate index streams per
    layer: layer 2 gathers from a window-concat AllGather layout.
  - device: SWDGE dma_gather of bf16 source rows round-robined over all 4
    SWDGE queues (4 Q7 core pairs generate descriptors in parallel);
    fused one-hot segment matrices on DVE; TensorE matmul msg^T @ seg
    accumulated per 512-node PSUM window = transposed mean-aggregation;
    1/deg folded into the PSUM evacuation; window tails (weight matmuls,
    relu, transposes, stores) software-pipelined one window behind the
    gather stream; per-window chunked AllGather of h overlaps layer-1
    compute; final layer emits row-major output directly.
"""

import os
import sys
import types

for _p in ("/opt/trn_rl_repo", "/root/.axon_site/_ro/trn_rl_repo",
           "/root/.axon_site"):
    if os.path.isdir(_p) and _p not in sys.path:
        sys.path.insert(0, _p)


def _install_ntff_hook():
    """Provide antenv.axon_hooks (missing in this image) so trace=True can
    capture NTFF profiles through libaxon_pjrt.so."""
    if "antenv.axon_hooks" in sys.modules:
        return
    store = [None]
    mod = types.ModuleType("antenv.axon_hooks")
    mod.set_axon_ntff_profile_hook = lambda h: store.__setitem__(0, h)
    mod.get_axon_ntff_profile_hook = lambda: store[0]
    sys.modules["antenv.axon_hooks"] = mod
    try:
        import antenv
        antenv.axon_hooks = mod
        from trn_agent_boot.trn_boot import _ntff_profile_via_ctypes
        so = "/opt/axon/libaxon_pjrt.so"
        if os.path.exists(so):
            mod.set_axon_ntff_profile_hook(_ntff_profile_via_ctypes(so))
    except Exception:
        pass


_install_ntff_hook()


import numpy as np
import ml_dtypes

import concourse.bass as bass
import concourse.bacc as bacc
import concourse.mybir as mybir
import concourse.tile as tile
from concourse.library_config import mlp as mlp_library

P = 128
D = 128
GMAX = 8  # max batches (1024 idxs) per dma_gather instruction
NQUEUES = 4  # SWDGE queues; queue k runs on Q7 core pair (2k, 2k+1)
HALF = 32768  # int16 index limit for dma_gather
F32 = mybir.dt.float32
BF16 = mybir.dt.bfloat16
I16 = mybir.dt.int16


def wrap_idx16(arr):
    """[n] int array -> [128, n//16] int16 SWDGE layout (16-partition wrap,
    replicated for the 8 Q7 cores)."""
    n = arr.shape[0]
    assert n % 16 == 0
    w = np.asarray(arr, dtype=np.int16).reshape(n // 16, 16).T  # [16, n/16]
    return np.tile(w, (8, 1))  # [128, n/16]


# ----------------------------------------------------------------- host prep
# AllGather chunking for h (local-row ranges; each chunk is one Shared
# tensor written by one collective). Rebased per-chunk indices must fit
# int16: ncores * chunk_len - 1 <= 32767.
CHUNK_R = [0, 4096, 5632, 6250]  # chunk k covers local rows [R[k], R[k+1])


def _build_stream(edges, ncores, nsub, nwin, spw, nph):
    """Equalize per-(sub, phase) batch counts across cores; build the
    blocks/idx16/slots stream (window-major, phases in order)."""
    nb = np.zeros((nsub, nph), dtype=np.int64)
    for c in range(ncores):
        for t in range(nsub):
            for ph in range(nph):
                n = len(edges[c][nph * t + ph][0])
                nb[t, ph] = max(nb[t, ph], (n + P - 1) // P)
    nb[:, 0] = np.maximum(nb[:, 0], 1)  # ensure each sub has >=1 batch

    blocks = []
    ncols = 0
    for w in range(nwin):
        subs = range(w * spw, min((w + 1) * spw, nsub))
        for ph in range(nph):
            bl = [(t, int(nb[t, ph])) for t in subs]
            nbl = sum(x[1] for x in bl)
            blocks.append(dict(w=w, ph=ph, col0=ncols, nb=nbl, subs=bl))
            ncols += nbl

    idx16 = np.zeros((ncores, P, ncols * 8), dtype=np.int16)
    slots = np.full((ncores, P, ncols), -1.0, dtype=ml_dtypes.bfloat16)
    for c in range(ncores):
        for blk in blocks:
            if blk["nb"] == 0:
                continue
            col = blk["col0"]
            flat_idx = []
            for t, nbt in blk["subs"]:
                s, d = edges[c][nph * t + blk["ph"]]
                n = len(s)
                npad = nbt * P
                si = np.zeros(npad, dtype=np.int64)
                si[:n] = s
                sl = np.full(npad, -1.0, dtype=np.float32)
                sl[:n] = (d % P).astype(np.float32)
                for b in range(nbt):
                    slots[c, :, col + b] = sl[b * P:(b + 1) * P].astype(
                        ml_dtypes.bfloat16)
                flat_idx.append(si)
                col += nbt
            fi = np.concatenate(flat_idx)
            idx16[c, :, blk["col0"] * 8:(blk["col0"] + blk["nb"]) * 8] = \
                wrap_idx16(fi)
    return dict(ncols=ncols, blocks=blocks, idx16=idx16, slots=slots, nph=nph)


def host_prep(edge_index, n_nodes, ncores, win=512):
    """Partition/sort/pad edges; build per-layer phased streams.

    Layer 0 gathers from x in natural node order (2 phases at the int16
    boundary). Layer 1 gathers from the chunked AllGather layout: global
    node (c, r) in chunk k lives at row c*len_k + (r - R[k]) of chunk
    tensor k (3 phases, one per chunk tensor)."""
    npc = n_nodes // ncores
    nsub = (npc + P - 1) // P
    nwin = (npc + win - 1) // win
    spw = win // P
    src_a = np.asarray(edge_index[0], dtype=np.int64)
    dst_a = np.asarray(edge_index[1], dtype=np.int64)

    c_of = src_a // npc
    r_of = src_a % npc
    nchunk = len(CHUNK_R) - 1
    chunk_of = np.searchsorted(CHUNK_R, r_of, side="right") - 1
    clen = np.diff(CHUNK_R)
    # rebased row within the chunk tensor
    rrow_a = c_of * clen[chunk_of] + (r_of - np.asarray(CHUNK_R)[chunk_of])

    invcnt = np.zeros((ncores, 1, npc), dtype=np.float32)
    per_core = []
    for c in range(ncores):
        lo_n = c * npc
        m = (dst_a >= lo_n) & (dst_a < lo_n + npc)
        d = dst_a[m] - lo_n
        invcnt[c, 0] = 1.0 / np.maximum(np.bincount(d, minlength=npc), 1.0)
        order = np.argsort(d, kind="stable")
        per_core.append((m, order, d[order]))

    # per-edge (phase, rebased idx) per layer
    ph0 = (src_a >= HALF).astype(np.int64)
    id0 = src_a - ph0 * HALF
    ph1 = chunk_of
    id1 = rrow_a

    layers = []
    for ph_a, id_a, nph in ((ph0, id0, 2), (ph1, id1, nchunk)):
        edges = [[None] * (nph * nsub) for _ in range(ncores)]
        for c in range(ncores):
            m, order, d_sorted = per_core[c]
            pe = ph_a[m][order]
            ie = id_a[m][order]
            sub = d_sorted // P
            for t in range(nsub):
                ms = sub == t
                for ph in range(nph):
                    mp = ms & (pe == ph)
                    edges[c][nph * t + ph] = (ie[mp], d_sorted[mp])
        layers.append(_build_stream(edges, ncores, nsub, nwin, spw, nph))

    return dict(npc=npc, nsub=nsub, nwin=nwin, win=win, invcnt=invcnt,
                layers=layers)


# -------------------------------------------------------------- kernel build
def build_kernel(n_nodes, ncores, prep, nb_onehot=8):
    npc, nwin, win = prep["npc"], prep["nwin"], prep["win"]
    L0, L1 = prep["layers"]
    spw = win // P

    nc = bacc.Bacc(None, num_swdge_queues=NQUEUES)

    xtab = nc.declare_dram_parameter("xtab", [n_nodes, D], BF16, isOutput=False)
    xT_d = nc.declare_dram_parameter("xT", [D, npc], F32, isOutput=False)
    idx0_d = nc.declare_dram_parameter("idx0", [P, L0["ncols"] * 8], I16, isOutput=False)
    seg0_d = nc.declare_dram_parameter("seg0", [P, L0["ncols"] * P], BF16, isOutput=False)
    idx1_d = nc.declare_dram_parameter("idx1", [P, L1["ncols"] * 8], I16, isOutput=False)
    seg1_d = nc.declare_dram_parameter("seg1", [P, L1["ncols"] * P], BF16, isOutput=False)
    invcnt_d = nc.declare_dram_parameter("invcnt", [P, npc], F32, isOutput=False)
    W1l_d = nc.declare_dram_parameter("W1l", [D, D], F32, isOutput=False)
    W1r_d = nc.declare_dram_parameter("W1r", [D, D], F32, isOutput=False)
    W2l_d = nc.declare_dram_parameter("W2l", [D, D], F32, isOutput=False)
    W2r_d = nc.declare_dram_parameter("W2r", [D, D], F32, isOutput=False)
    b1_d = nc.declare_dram_parameter("b1", [D, 1], F32, isOutput=False)
    b2row_d = nc.declare_dram_parameter("b2row", [P, D], F32, isOutput=False)
    ident_d = nc.declare_dram_parameter("ident", [P, P], F32, isOutput=False)
    out_d = nc.declare_dram_parameter("out", [npc, D], F32, isOutput=True)

    from contextlib import ExitStack
    with tile.TileContext(nc) as tc, ExitStack() as es:
        dram = es.enter_context(tc.tile_pool(name="dram", bufs=1, space="DRAM"))
        h_local = dram.tile([npc, D], BF16, tag="hloc")
        # AllGather chunk tensors (each written by exactly one collective)
        h_chunks = []
        for k in range(len(CHUNK_R) - 1):
            clen = CHUNK_R[k + 1] - CHUNK_R[k]
            hck = dram.tile([ncores * clen, D], BF16, tag=f"hck{k}",
                            name=f"hck{k}", addr_space="Shared")
            h_chunks.append(hck)

        const = es.enter_context(tc.tile_pool(name="const", bufs=1))
        sb = es.enter_context(tc.tile_pool(name="sb", bufs=1))
        msgp = es.enter_context(tc.tile_pool(name="msgp", bufs=16))
        segp = es.enter_context(tc.tile_pool(name="segp", bufs=10))
        aggp = es.enter_context(tc.tile_pool(name="aggp", bufs=3))
        rowp = es.enter_context(tc.tile_pool(name="rowp", bufs=4))
        psA = es.enter_context(tc.tile_pool(name="psA", bufs=3, space="PSUM"))
        psB = es.enter_context(tc.tile_pool(name="psB", bufs=2, space="PSUM"))
        psT = es.enter_context(tc.tile_pool(name="psT", bufs=3, space="PSUM"))

        nc.gpsimd.load_library(mlp_library)

        idx0_sb = const.tile([P, L0["ncols"] * 8], I16, tag="idx0")
        idx1_sb = const.tile([P, L1["ncols"] * 8], I16, tag="idx1")
        invcnt_sb = const.tile([P, npc], F32, tag="invcnt")
        ident_sb = const.tile([P, P], F32, tag="ident")
        W1l_sb = const.tile([D, D], F32, tag="W1l")
        W1r_sb = const.tile([D, D], F32, tag="W1r")
        W2l_sb = const.tile([D, D], F32, tag="W2l")
        W2r_sb = const.tile([D, D], F32, tag="W2r")
        b1_sb = const.tile([D, 1], F32, tag="b1")
        b2row_sb = const.tile([P, D], F32, tag="b2row")
        xT_sb = sb.tile([D, npc], F32, tag="xT")
        hT_sb = sb.tile([D, npc], F32, tag="hT")

        # gather-critical loads first on the Sync HWDGE queue; the rest on
        # the Scalar HWDGE queue so the two rings drain in parallel
        for t, dd in [(idx0_sb, idx0_d), (W1l_sb, W1l_d), (W1r_sb, W1r_d),
                      (b1_sb, b1_d), (xT_sb, xT_d)]:
            nc.sync.dma_start(out=t[:], in_=dd[:])
        for t, dd in [(idx1_sb, idx1_d), (invcnt_sb, invcnt_d),
                      (ident_sb, ident_d), (W2l_sb, W2l_d), (W2r_sb, W2r_d),
                      (b2row_sb, b2row_d)]:
            nc.scalar.dma_start(out=t[:], in_=dd[:])

        gq = [0]  # round-robin SWDGE queue counter (4 Q7 core pairs)

        def emit_cc(k):
            r0, r1 = CHUNK_R[k], CHUNK_R[k + 1]
            nc.gpsimd.collective_compute(
                "AllGather", mybir.AluOpType.bypass,
                replica_groups=[list(range(ncores))],
                ins=[h_local[r0:r1, :]],
                outs=[h_chunks[k][:]])

        # emit chunk k's collective right after the tail of its last window:
        # the Pool sequencer reaches it while that window's stores land, so
        # the collective triggers as early as the in-order stream allows
        cc_after_tail = {}
        for k in range(len(CHUNK_R) - 1):
            last_w = (CHUNK_R[k + 1] - 1) // win
            cc_after_tail[last_w] = k
        cc_at_end = [k for w, k in cc_after_tail.items() if w >= nwin - 1]
        cc_after_tail = {w: k for w, k in cc_after_tail.items()
                         if w < nwin - 1}

        def emit_layer(layer, lp, tabs, idx_sb, seg_d, chunked_cc,
                       cc_pending=()):
            cc_pending = set(cc_pending)
            ncols, blocks = lp["ncols"], lp["blocks"]
            ngrp = (ncols + nb_onehot - 1) // nb_onehot
            segs = []
            for g in range(ngrp):
                nbg = min(nb_onehot, ncols - g * nb_onehot)
                seg = segp.tile([P, nb_onehot, P], BF16, tag="seg",
                                name=f"seg{layer}_{g}")
                g0 = g * nb_onehot
                nc.sync.dma_start(
                    out=seg[:, :nbg, :],
                    in_=seg_d[:, g0 * P:(g0 + nbg) * P])
                segs.append(seg)

            def tail(w, agg_ps):
                n0 = w * win
                wn = min(win, npc - n0)
                nsw = (wn + P - 1) // P
                aggTs = aggp.tile([P, win], F32, tag="aggTs",
                                  name=f"aggTs{layer}_{w}")
                nc.vector.tensor_tensor(
                    out=aggTs[:, :wn], in0=agg_ps[:, :wn],
                    in1=invcnt_sb[:, n0:n0 + wn], op=mybir.AluOpType.mult)

                if layer == 0:
                    ab_ps = psB.tile([P, win], F32, tag="AB", name=f"ab{w}")
                    nc.tensor.matmul(out=ab_ps[:, :wn], lhsT=W1l_sb[:],
                                     rhs=aggTs[:, :wn], start=True, stop=False)
                    nc.tensor.matmul(out=ab_ps[:, :wn], lhsT=W1r_sb[:],
                                     rhs=xT_sb[:, n0:n0 + wn], start=False,
                                     stop=True)
                    nc.scalar.activation(
                        out=hT_sb[:, n0:n0 + wn], in_=ab_ps[:, :wn],
                        func=mybir.ActivationFunctionType.Relu,
                        bias=b1_sb[:, 0:1], scale=1.0)
                    for j in range(nsw):
                        r0 = n0 + j * P
                        ns = min(P, npc - r0)
                        tr_ps = psT.tile([P, P], F32, tag="tr",
                                         name=f"tr{w}_{j}")
                        nc.tensor.transpose(out=tr_ps[:ns, :],
                                            in_=hT_sb[:, r0:r0 + ns],
                                            identity=ident_sb[:])
                        hrow = rowp.tile([P, D], BF16, tag="hrow",
                                         name=f"hrow{w}_{j}")
                        nc.scalar.activation(
                            out=hrow[:ns, :], in_=tr_ps[:ns, :],
                            func=mybir.ActivationFunctionType.Copy, scale=1.0)
                        nc.sync.dma_start(out=h_local[r0:r0 + ns, :],
                                          in_=hrow[:ns, :])
                else:
                    for j in range(nsw):
                        r0 = n0 + j * P
                        ns = min(P, npc - r0)
                        o_ps = psT.tile([P, P], F32, tag="tr",
                                        name=f"ops{w}_{j}")
                        nc.tensor.matmul(out=o_ps[:ns, :],
                                         lhsT=aggTs[:, j * P:j * P + ns],
                                         rhs=W2l_sb[:], start=True, stop=False)
                        nc.tensor.matmul(out=o_ps[:ns, :],
                                         lhsT=hT_sb[:, r0:r0 + ns],
                                         rhs=W2r_sb[:], start=False, stop=True)
                        orow = rowp.tile([P, D], F32, tag="orow",
                                         name=f"orow{w}_{j}")
                        nc.vector.tensor_tensor(
                            out=orow[:ns, :], in0=o_ps[:ns, :],
                            in1=b2row_sb[:ns, :], op=mybir.AluOpType.add)
                        nc.sync.dma_start(out=out_d[r0:r0 + ns, :],
                                          in_=orow[:ns, :])

            prev = None  # software-pipelined tail: emitted one window late
            for w in range(nwin):
                agg_ps = psA.tile([P, win], F32, tag="aggT",
                                  name=f"agg{layer}_{w}")
                sub_of_b = {}
                for blk in blocks:
                    if blk["w"] != w or blk["nb"] == 0:
                        continue
                    col = blk["col0"]
                    for t, nbt in blk["subs"]:
                        for bi in range(nbt):
                            sub_of_b[col + bi] = t
                        col += nbt
                win_first_b = min(sub_of_b)
                win_last_b = max(sub_of_b)

                for blk in blocks:
                    if blk["w"] != w or blk["nb"] == 0:
                        continue
                    if blk["ph"] in cc_pending:
                        # deferred final AllGather chunk: emitted just before
                        # the first gather that reads it, so earlier-phase
                        # gathers dispatch without waiting on layer-1 stores
                        emit_cc(blk["ph"])
                        cc_pending.discard(blk["ph"])
                    tab = tabs[blk["ph"]]
                    for c0 in range(0, blk["nb"], GMAX):
                        cn = min(GMAX, blk["nb"] - c0)
                        msg = msgp.tile([P, GMAX, D], BF16, tag="msg",
                                        name=f"msg{layer}_{w}_{blk['ph']}_{c0}")
                        nidx = cn * P
                        b0 = blk["col0"] + c0
                        nc.gpsimd.dma_gather(
                            out_ap=msg[:, :cn, :],
                            in_ap=tab,
                            idxs_ap=idx_sb[:, b0 * 8:(b0 + cn) * 8],
                            num_idxs=nidx,
                            num_idxs_reg=nidx,
                            elem_size=D,
                            queue_num=gq[0] % NQUEUES,
                        )
                        gq[0] += 1
                        for bi in range(cn):
                            b = b0 + bi
                            t = sub_of_b[b]
                            j = t - w * spw
                            nsl = min(P, npc - t * P)
                            nc.tensor.matmul(
                                out=agg_ps[:, j * P:j * P + nsl],
                                lhsT=msg[:, bi, :],
                                rhs=segs[b // nb_onehot][:, b % nb_onehot, :nsl],
                                start=(b == win_first_b),
                                stop=(b == win_last_b),
                            )

                if prev is not None:
                    tail(*prev)
                    if chunked_cc and prev[0] in cc_after_tail:
                        emit_cc(cc_after_tail[prev[0]])
                prev = (w, agg_ps)

            tail(*prev)
            if chunked_cc:
                if prev[0] in cc_after_tail:
                    emit_cc(cc_after_tail[prev[0]])
            for k in sorted(cc_pending):
                emit_cc(k)

        emit_layer(0, L0, [xtab[0:HALF, :], xtab[HALF:n_nodes, :]],
                   idx0_sb, seg0_d, chunked_cc=True)
        emit_layer(1, L1, [t[:] for t in h_chunks],
                   idx1_sb, seg1_d, chunked_cc=False,
                   cc_pending=cc_at_end)

    nc.finalize()
    return nc


# ---------------------------------------------------------------- in_maps
def make_in_maps(x, edge_index, W1_l, b1_l, W1_r, W2_l, b2_l, W2_r,
                 n_nodes, ncores, win=512):
    prep = host_prep(edge_index, n_nodes, ncores, win=win)
    npc = prep["npc"]
    L0, L1 = prep["layers"]
    x = np.asarray(x, dtype=np.float32)
    xtab = x.astype(ml_dtypes.bfloat16)
    xT = np.ascontiguousarray(x.T)
    ident = np.eye(P, dtype=np.float32)
    common = dict(
        xtab=xtab,
        W1l=np.asarray(W1_l, np.float32), W1r=np.asarray(W1_r, np.float32),
        W2l=np.asarray(W2_l, np.float32), W2r=np.asarray(W2_r, np.float32),
        b1=np.asarray(b1_l, np.float32).reshape(D, 1),
        b2row=np.tile(np.asarray(b2_l, np.float32).reshape(1, D), (P, 1)),
        ident=ident,
    )
    arange = np.arange(P, dtype=np.float32)[None, None, :]

    def seg_host(lp, c):
        # [P, ncols*P] bf16 one-hot: col g0*P + b*P + f = (slots[p, g0+b]==f)
        sl = np.asarray(lp["slots"][c], dtype=np.float32)  # [P, ncols]
        oh = (sl[:, :, None] == arange).astype(ml_dtypes.bfloat16)
        return np.ascontiguousarray(oh.reshape(P, -1))

    in_maps = []
    for c in range(ncores):
        in_maps.append(dict(
            common,
            xT=np.ascontiguousarray(xT[:, c * npc:(c + 1) * npc]),
            idx0=L0["idx16"][c], seg0=seg_host(L0, c),
            idx1=L1["idx16"][c], seg1=seg_host(L1, c),
            invcnt=np.tile(prep["invcnt"][c], (P, 1)),
        ))
    return prep, in_maps


# ------------------------------------------------------------------ kernel()
N_NODES = 50000
NCORES = 8

_cache = {}
last_result = None  # BassKernelResults of the most recent run (for test.py)


def kernel(x, edge_index, W1_l, b1_l, W1_r, W2_l, b2_l, W2_r,
           trace=False, trace_kwargs=None):
    """Full inputs in, full output out. Shards across 8 NeuronCores."""
    global last_result
    from concourse.bass_utils import run_bass_kernel_spmd

    x = np.asarray(x)
    edge_index = np.asarray(edge_index)
    n_nodes = x.shape[0]
    assert n_nodes % NCORES == 0

    prep, in_maps = make_in_maps(x, edge_index, W1_l, b1_l, W1_r,
                                 W2_l, b2_l, W2_r, n_nodes, NCORES)
    key = (n_nodes,) + tuple(
        (lp["ncols"],) + tuple(blk["nb"] for blk in lp["blocks"])
        for lp in prep["layers"])
    if key not in _cache:
        _cache[key] = build_kernel(n_nodes, NCORES, prep)
    nc = _cache[key]

    res = run_bass_kernel_spmd(nc, in_maps, list(range(NCORES)),
                               trace=trace, **(trace_kwargs or {}))
    last_result = res
    out = np.concatenate([res.results[c]["out"] for c in range(NCORES)],
                         axis=0)
    return out.astype(np.float32)


# revision 32
# speedup vs baseline: 1.1308x; 1.1308x over previous
"""Trainium2 Bass kernel: 2-layer GraphSAGE (mean aggregation), 8-core SPMD.

nn_BiGNN: out = sage2(relu(sage1(x)));  sage(x) = mean_{j->i}(x_j) @ W_l + b_l + x @ W_r
N=50000 nodes, E=800000 edges, d=128, f32 inputs / f32 output.

Strategy (one NeuronCore owns 6250 destination nodes):
  - host: partition edges by destination block, sort by dst, pad per
    128-dst subwindow, split into lo/hi phases (int16 SWDGE index limit),
    equalize batch counts across cores (SPMD). Separate index streams per
    layer: layer 2 gathers from a window-concat AllGather layout.
  - device: SWDGE dma_gather of bf16 source rows round-robined over all 4
    SWDGE queues (4 Q7 core pairs generate descriptors in parallel);
    fused one-hot segment matrices on DVE; TensorE matmul msg^T @ seg
    accumulated per 512-node PSUM window = transposed mean-aggregation;
    1/deg folded into the PSUM evacuation; window tails (weight matmuls,
    relu, transposes, stores) software-pipelined one window behind the
    gather stream; per-window chunked AllGather of h overlaps layer-1
    compute; final layer emits row-major output directly.
"""

import os
import sys
import types

for _p in ("/opt/trn_rl_repo", "/root/.axon_site/_ro/trn_rl_repo",
           "/root/.axon_site"):
    if os.path.isdir(_p) and _p not in sys.path:
        sys.path.insert(0, _p)


def _install_ntff_hook():
    """Provide antenv.axon_hooks (missing in this image) so trace=True can
    capture NTFF profiles through libaxon_pjrt.so."""
    if "antenv.axon_hooks" in sys.modules:
        return
    store = [None]
    mod = types.ModuleType("antenv.axon_hooks")
    mod.set_axon_ntff_profile_hook = lambda h: store.__setitem__(0, h)
    mod.get_axon_ntff_profile_hook = lambda: store[0]
    sys.modules["antenv.axon_hooks"] = mod
    try:
        import antenv
        antenv.axon_hooks = mod
        from trn_agent_boot.trn_boot import _ntff_profile_via_ctypes
        so = "/opt/axon/libaxon_pjrt.so"
        if os.path.exists(so):
            mod.set_axon_ntff_profile_hook(_ntff_profile_via_ctypes(so))
    except Exception:
        pass


_install_ntff_hook()


import numpy as np
import ml_dtypes

import concourse.bass as bass
import concourse.bacc as bacc
import concourse.mybir as mybir
import concourse.tile as tile
from concourse.library_config import mlp as mlp_library

P = 128
D = 128
GMAX = 8  # max batches (1024 idxs) per dma_gather instruction (ucode limit)
NQUEUES = 4  # SWDGE queues; queue k runs on Q7 core pair (2k, 2k+1)
HALF = 32768  # int16 index limit for dma_gather
F32 = mybir.dt.float32
BF16 = mybir.dt.bfloat16
I16 = mybir.dt.int16


def wrap_idx16(arr):
    """[n] int array -> [128, n//16] int16 SWDGE layout (16-partition wrap,
    replicated for the 8 Q7 cores)."""
    n = arr.shape[0]
    assert n % 16 == 0
    w = np.asarray(arr, dtype=np.int16).reshape(n // 16, 16).T  # [16, n/16]
    return np.tile(w, (8, 1))  # [128, n/16]


# ----------------------------------------------------------------- host prep
# AllGather chunking for h (local-row ranges; each chunk is one Shared
# tensor written by one collective). Rebased per-chunk indices must fit
# int16: ncores * chunk_len - 1 <= 32767.
CHUNK_R = [0, 4096, 5632, 6250]  # chunk k covers local rows [R[k], R[k+1])


def _build_stream(edges, ncores, nsub, nwin, spw, nph):
    """Equalize per-(sub, phase) batch counts across cores; build the
    blocks/idx16/slots stream (window-major, phases in order)."""
    nb = np.zeros((nsub, nph), dtype=np.int64)
    for c in range(ncores):
        for t in range(nsub):
            for ph in range(nph):
                n = len(edges[c][nph * t + ph][0])
                nb[t, ph] = max(nb[t, ph], (n + P - 1) // P)
    nb[:, 0] = np.maximum(nb[:, 0], 1)  # ensure each sub has >=1 batch

    blocks = []
    ncols = 0
    for w in range(nwin):
        subs = range(w * spw, min((w + 1) * spw, nsub))
        for ph in range(nph):
            bl = [(t, int(nb[t, ph])) for t in subs]
            nbl = sum(x[1] for x in bl)
            blocks.append(dict(w=w, ph=ph, col0=ncols, nb=nbl, subs=bl))
            ncols += nbl

    idx16 = np.zeros((ncores, P, ncols * 8), dtype=np.int16)
    slots = np.full((ncores, P, ncols), -1.0, dtype=ml_dtypes.bfloat16)
    for c in range(ncores):
        for blk in blocks:
            if blk["nb"] == 0:
                continue
            col = blk["col0"]
            flat_idx = []
            for t, nbt in blk["subs"]:
                s, d = edges[c][nph * t + blk["ph"]]
                n = len(s)
                npad = nbt * P
                si = np.zeros(npad, dtype=np.int64)
                si[:n] = s
                sl = np.full(npad, -1.0, dtype=np.float32)
                sl[:n] = (d % P).astype(np.float32)
                for b in range(nbt):
                    slots[c, :, col + b] = sl[b * P:(b + 1) * P].astype(
                        ml_dtypes.bfloat16)
                flat_idx.append(si)
                col += nbt
            fi = np.concatenate(flat_idx)
            idx16[c, :, blk["col0"] * 8:(blk["col0"] + blk["nb"]) * 8] = \
                wrap_idx16(fi)
    return dict(ncols=ncols, blocks=blocks, idx16=idx16, slots=slots, nph=nph)


def host_prep(edge_index, n_nodes, ncores, win=512):
    """Partition/sort/pad edges; build per-layer phased streams.

    Layer 0 gathers from x in natural node order (2 phases at the int16
    boundary). Layer 1 gathers from the chunked AllGather layout: global
    node (c, r) in chunk k lives at row c*len_k + (r - R[k]) of chunk
    tensor k (3 phases, one per chunk tensor)."""
    npc = n_nodes // ncores
    nsub = (npc + P - 1) // P
    nwin = (npc + win - 1) // win
    spw = win // P
    src_a = np.asarray(edge_index[0], dtype=np.int64)
    dst_a = np.asarray(edge_index[1], dtype=np.int64)

    c_of = src_a // npc
    r_of = src_a % npc
    nchunk = len(CHUNK_R) - 1
    chunk_of = np.searchsorted(CHUNK_R, r_of, side="right") - 1
    clen = np.diff(CHUNK_R)
    # rebased row within the chunk tensor
    rrow_a = c_of * clen[chunk_of] + (r_of - np.asarray(CHUNK_R)[chunk_of])

    invcnt = np.zeros((ncores, 1, npc), dtype=np.float32)
    per_core = []
    for c in range(ncores):
        lo_n = c * npc
        m = (dst_a >= lo_n) & (dst_a < lo_n + npc)
        d = dst_a[m] - lo_n
        invcnt[c, 0] = 1.0 / np.maximum(np.bincount(d, minlength=npc), 1.0)
        order = np.argsort(d, kind="stable")
        per_core.append((m, order, d[order]))

    # per-edge (phase, rebased idx) per layer
    ph0 = (src_a >= HALF).astype(np.int64)
    id0 = src_a - ph0 * HALF
    ph1 = chunk_of
    id1 = rrow_a

    layers = []
    for ph_a, id_a, nph in ((ph0, id0, 2), (ph1, id1, nchunk)):
        edges = [[None] * (nph * nsub) for _ in range(ncores)]
        for c in range(ncores):
            m, order, d_sorted = per_core[c]
            pe = ph_a[m][order]
            ie = id_a[m][order]
            sub = d_sorted // P
            for t in range(nsub):
                ms = sub == t
                for ph in range(nph):
                    mp = ms & (pe == ph)
                    edges[c][nph * t + ph] = (ie[mp], d_sorted[mp])
        layers.append(_build_stream(edges, ncores, nsub, nwin, spw, nph))

    return dict(npc=npc, nsub=nsub, nwin=nwin, win=win, invcnt=invcnt,
                layers=layers)


# -------------------------------------------------------------- kernel build
def build_kernel(n_nodes, ncores, prep, nb_onehot=8):
    npc, nwin, win = prep["npc"], prep["nwin"], prep["win"]
    L0, L1 = prep["layers"]
    spw = win // P

    nc = bacc.Bacc(None, num_swdge_queues=NQUEUES)

    xtab = nc.declare_dram_parameter("xtab", [n_nodes, D], BF16, isOutput=False)
    xT_d = nc.declare_dram_parameter("xT", [D, npc], F32, isOutput=False)
    idx0_d = nc.declare_dram_parameter("idx0", [P, L0["ncols"] * 8], I16, isOutput=False)
    slots0_d = nc.declare_dram_parameter("slots0", [P, L0["ncols"]], BF16, isOutput=False)
    idx1_d = nc.declare_dram_parameter("idx1", [P, L1["ncols"] * 8], I16, isOutput=False)
    slots1_d = nc.declare_dram_parameter("slots1", [P, L1["ncols"]], BF16, isOutput=False)
    invcnt_d = nc.declare_dram_parameter("invcnt", [P, npc], F32, isOutput=False)
    W1l_d = nc.declare_dram_parameter("W1l", [D, D], F32, isOutput=False)
    W1r_d = nc.declare_dram_parameter("W1r", [D, D], F32, isOutput=False)
    W2l_d = nc.declare_dram_parameter("W2l", [D, D], F32, isOutput=False)
    W2r_d = nc.declare_dram_parameter("W2r", [D, D], F32, isOutput=False)
    b1_d = nc.declare_dram_parameter("b1", [D, 1], F32, isOutput=False)
    b2row_d = nc.declare_dram_parameter("b2row", [P, D], F32, isOutput=False)
    iota_d = nc.declare_dram_parameter("iota", [P, P], BF16, isOutput=False)
    ident_d = nc.declare_dram_parameter("ident", [P, P], F32, isOutput=False)
    out_d = nc.declare_dram_parameter("out", [npc, D], F32, isOutput=True)

    from contextlib import ExitStack
    with tile.TileContext(nc) as tc, ExitStack() as es:
        dram = es.enter_context(tc.tile_pool(name="dram", bufs=1, space="DRAM"))
        h_local = dram.tile([npc, D], BF16, tag="hloc")
        # AllGather chunk tensors (each written by exactly one collective)
        h_chunks = []
        for k in range(len(CHUNK_R) - 1):
            clen = CHUNK_R[k + 1] - CHUNK_R[k]
            hck = dram.tile([ncores * clen, D], BF16, tag=f"hck{k}",
                            name=f"hck{k}", addr_space="Shared")
            h_chunks.append(hck)

        const = es.enter_context(tc.tile_pool(name="const", bufs=1))
        sb = es.enter_context(tc.tile_pool(name="sb", bufs=1))
        msgp = es.enter_context(tc.tile_pool(name="msgp", bufs=16))
        segp = es.enter_context(tc.tile_pool(name="segp", bufs=10))
        aggp = es.enter_context(tc.tile_pool(name="aggp", bufs=3))
        rowp = es.enter_context(tc.tile_pool(name="rowp", bufs=4))
        psA = es.enter_context(tc.tile_pool(name="psA", bufs=3, space="PSUM"))
        psB = es.enter_context(tc.tile_pool(name="psB", bufs=2, space="PSUM"))
        psT = es.enter_context(tc.tile_pool(name="psT", bufs=3, space="PSUM"))

        nc.gpsimd.load_library(mlp_library)

        idx0_sb = const.tile([P, L0["ncols"] * 8], I16, tag="idx0")
        slots0_sb = const.tile([P, L0["ncols"]], BF16, tag="slots0")
        idx1_sb = const.tile([P, L1["ncols"] * 8], I16, tag="idx1")
        slots1_sb = const.tile([P, L1["ncols"]], BF16, tag="slots1")
        invcnt_sb = const.tile([P, npc], F32, tag="invcnt")
        iota_sb = const.tile([P, P], BF16, tag="iota")
        ident_sb = const.tile([P, P], F32, tag="ident")
        W1l_sb = const.tile([D, D], F32, tag="W1l")
        W1r_sb = const.tile([D, D], F32, tag="W1r")
        W2l_sb = const.tile([D, D], F32, tag="W2l")
        W2r_sb = const.tile([D, D], F32, tag="W2r")
        b1_sb = const.tile([D, 1], F32, tag="b1")
        b2row_sb = const.tile([P, D], F32, tag="b2row")
        xT_sb = sb.tile([D, npc], F32, tag="xT")
        hT_sb = sb.tile([D, npc], F32, tag="hT")

        # gather-critical loads first on the Sync HWDGE queue; the rest on
        # the Scalar HWDGE queue so the two rings drain in parallel
        for t, dd in [(idx0_sb, idx0_d), (slots0_sb, slots0_d),
                      (iota_sb, iota_d), (W1l_sb, W1l_d), (W1r_sb, W1r_d),
                      (b1_sb, b1_d), (xT_sb, xT_d)]:
            nc.sync.dma_start(out=t[:], in_=dd[:])
        for t, dd in [(idx1_sb, idx1_d), (slots1_sb, slots1_d),
                      (invcnt_sb, invcnt_d), (ident_sb, ident_d),
                      (W2l_sb, W2l_d), (W2r_sb, W2r_d), (b2row_sb, b2row_d)]:
            nc.scalar.dma_start(out=t[:], in_=dd[:])

        gq = [0]  # round-robin SWDGE queue counter (4 Q7 core pairs)

        def emit_cc(k):
            r0, r1 = CHUNK_R[k], CHUNK_R[k + 1]
            nc.gpsimd.collective_compute(
                "AllGather", mybir.AluOpType.bypass,
                replica_groups=[list(range(ncores))],
                ins=[h_local[r0:r1, :]],
                outs=[h_chunks[k][:]])

        # emit chunk k's collective right after the tail of its last window:
        # the Pool sequencer reaches it while that window's stores land, so
        # the collective triggers as early as the in-order stream allows
        cc_after_tail = {}
        for k in range(len(CHUNK_R) - 1):
            last_w = (CHUNK_R[k + 1] - 1) // win
            cc_after_tail[last_w] = k
        cc_at_end = [k for w, k in cc_after_tail.items() if w >= nwin - 1]
        cc_after_tail = {w: k for w, k in cc_after_tail.items()
                         if w < nwin - 1}

        def emit_layer(layer, lp, tabs, idx_sb, slots_sb, chunked_cc,
                       cc_pending=()):
            cc_pending = set(cc_pending)
            ncols, blocks = lp["ncols"], lp["blocks"]
            ngrp = (ncols + nb_onehot - 1) // nb_onehot
            segs = []
            for g in range(ngrp):
                nbg = min(nb_onehot, ncols - g * nb_onehot)
                seg = segp.tile([P, nb_onehot, P], BF16, tag="seg",
                                name=f"seg{layer}_{g}")
                g0 = g * nb_onehot
                nc.vector.tensor_tensor(
                    out=seg[:, :nbg, :],
                    in0=iota_sb[:, None, :].to_broadcast([P, nbg, P]),
                    in1=slots_sb[:, g0:g0 + nbg, None].to_broadcast([P, nbg, P]),
                    op=mybir.AluOpType.is_equal,
                )
                segs.append(seg)

            def tail(w, agg_ps):
                n0 = w * win
                wn = min(win, npc - n0)
                nsw = (wn + P - 1) // P
                aggTs = aggp.tile([P, win], F32, tag="aggTs",
                                  name=f"aggTs{layer}_{w}")
                nc.vector.tensor_tensor(
                    out=aggTs[:, :wn], in0=agg_ps[:, :wn],
                    in1=invcnt_sb[:, n0:n0 + wn], op=mybir.AluOpType.mult)

                if layer == 0:
                    ab_ps = psB.tile([P, win], F32, tag="AB", name=f"ab{w}")
                    nc.tensor.matmul(out=ab_ps[:, :wn], lhsT=W1l_sb[:],
                                     rhs=aggTs[:, :wn], start=True, stop=False)
                    nc.tensor.matmul(out=ab_ps[:, :wn], lhsT=W1r_sb[:],
                                     rhs=xT_sb[:, n0:n0 + wn], start=False,
                                     stop=True)
                    nc.scalar.activation(
                        out=hT_sb[:, n0:n0 + wn], in_=ab_ps[:, :wn],
                        func=mybir.ActivationFunctionType.Relu,
                        bias=b1_sb[:, 0:1], scale=1.0)
                    for j in range(nsw):
                        r0 = n0 + j * P
                        ns = min(P, npc - r0)
                        tr_ps = psT.tile([P, P], F32, tag="tr",
                                         name=f"tr{w}_{j}")
                        nc.tensor.transpose(out=tr_ps[:ns, :],
                                            in_=hT_sb[:, r0:r0 + ns],
                                            identity=ident_sb[:])
                        hrow = rowp.tile([P, D], BF16, tag="hrow",
                                         name=f"hrow{w}_{j}")
                        nc.scalar.activation(
                            out=hrow[:ns, :], in_=tr_ps[:ns, :],
                            func=mybir.ActivationFunctionType.Copy, scale=1.0)
                        nc.sync.dma_start(out=h_local[r0:r0 + ns, :],
                                          in_=hrow[:ns, :])
                else:
                    for j in range(nsw):
                        r0 = n0 + j * P
                        ns = min(P, npc - r0)
                        o_ps = psT.tile([P, P], F32, tag="tr",
                                        name=f"ops{w}_{j}")
                        nc.tensor.matmul(out=o_ps[:ns, :],
                                         lhsT=aggTs[:, j * P:j * P + ns],
                                         rhs=W2l_sb[:], start=True, stop=False)
                        nc.tensor.matmul(out=o_ps[:ns, :],
                                         lhsT=hT_sb[:, r0:r0 + ns],
                                         rhs=W2r_sb[:], start=False, stop=True)
                        orow = rowp.tile([P, D], F32, tag="orow",
                                         name=f"orow{w}_{j}")
                        nc.vector.tensor_tensor(
                            out=orow[:ns, :], in0=o_ps[:ns, :],
                            in1=b2row_sb[:ns, :], op=mybir.AluOpType.add)
                        nc.sync.dma_start(out=out_d[r0:r0 + ns, :],
                                          in_=orow[:ns, :])

            prev = None  # software-pipelined tail: emitted one window late
            for w in range(nwin):
                agg_ps = psA.tile([P, win], F32, tag="aggT",
                                  name=f"agg{layer}_{w}")
                sub_of_b = {}
                for blk in blocks:
                    if blk["w"] != w or blk["nb"] == 0:
                        continue
                    col = blk["col0"]
                    for t, nbt in blk["subs"]:
                        for bi in range(nbt):
                            sub_of_b[col + bi] = t
                        col += nbt
                win_first_b = min(sub_of_b)
                win_last_b = max(sub_of_b)

                for blk in blocks:
                    if blk["w"] != w or blk["nb"] == 0:
                        continue
                    if blk["ph"] in cc_pending:
                        # deferred final AllGather chunk: emitted just before
                        # the first gather that reads it, so earlier-phase
                        # gathers dispatch without waiting on layer-1 stores
                        emit_cc(blk["ph"])
                        cc_pending.discard(blk["ph"])
                    tab = tabs[blk["ph"]]
                    for c0 in range(0, blk["nb"], GMAX):
                        cn = min(GMAX, blk["nb"] - c0)
                        msg = msgp.tile([P, GMAX, D], BF16, tag="msg",
                                        name=f"msg{layer}_{w}_{blk['ph']}_{c0}")
                        nidx = cn * P
                        b0 = blk["col0"] + c0
                        nc.gpsimd.dma_gather(
                            out_ap=msg[:, :cn, :],
                            in_ap=tab,
                            idxs_ap=idx_sb[:, b0 * 8:(b0 + cn) * 8],
                            num_idxs=nidx,
                            num_idxs_reg=nidx,
                            elem_size=D,
                            queue_num=gq[0] % NQUEUES,
                        )
                        gq[0] += 1
                        for bi in range(cn):
                            b = b0 + bi
                            t = sub_of_b[b]
                            j = t - w * spw
                            nsl = min(P, npc - t * P)
                            nc.tensor.matmul(
                                out=agg_ps[:, j * P:j * P + nsl],
                                lhsT=msg[:, bi, :],
                                rhs=segs[b // nb_onehot][:, b % nb_onehot, :nsl],
                                start=(b == win_first_b),
                                stop=(b == win_last_b),
                            )

                if prev is not None:
                    tail(*prev)
                    if chunked_cc and prev[0] in cc_after_tail:
                        emit_cc(cc_after_tail[prev[0]])
                prev = (w, agg_ps)

            tail(*prev)
            if chunked_cc:
                if prev[0] in cc_after_tail:
                    emit_cc(cc_after_tail[prev[0]])
            for k in sorted(cc_pending):
                emit_cc(k)

        emit_layer(0, L0, [xtab[0:HALF, :], xtab[HALF:n_nodes, :]],
                   idx0_sb, slots0_sb, chunked_cc=True)
        emit_layer(1, L1, [t[:] for t in h_chunks],
                   idx1_sb, slots1_sb, chunked_cc=False,
                   cc_pending=cc_at_end)

    nc.finalize()
    return nc


# ---------------------------------------------------------------- in_maps
def make_in_maps(x, edge_index, W1_l, b1_l, W1_r, W2_l, b2_l, W2_r,
                 n_nodes, ncores, win=512):
    prep = host_prep(edge_index, n_nodes, ncores, win=win)
    npc = prep["npc"]
    L0, L1 = prep["layers"]
    x = np.asarray(x, dtype=np.float32)
    xtab = x.astype(ml_dtypes.bfloat16)
    xT = np.ascontiguousarray(x.T)
    iota = np.tile(np.arange(P, dtype=np.float32)[None, :], (P, 1)).astype(
        ml_dtypes.bfloat16)
    ident = np.eye(P, dtype=np.float32)
    common = dict(
        xtab=xtab,
        W1l=np.asarray(W1_l, np.float32), W1r=np.asarray(W1_r, np.float32),
        W2l=np.asarray(W2_l, np.float32), W2r=np.asarray(W2_r, np.float32),
        b1=np.asarray(b1_l, np.float32).reshape(D, 1),
        b2row=np.tile(np.asarray(b2_l, np.float32).reshape(1, D), (P, 1)),
        iota=iota, ident=ident,
    )
    in_maps = []
    for c in range(ncores):
        in_maps.append(dict(
            common,
            xT=np.ascontiguousarray(xT[:, c * npc:(c + 1) * npc]),
            idx0=L0["idx16"][c], slots0=L0["slots"][c],
            idx1=L1["idx16"][c], slots1=L1["slots"][c],
            invcnt=np.tile(prep["invcnt"][c], (P, 1)),
        ))
    return prep, in_maps


# ------------------------------------------------------------------ kernel()
N_NODES = 50000
NCORES = 8

_cache = {}
last_result = None  # BassKernelResults of the most recent run (for test.py)


def kernel(x, edge_index, W1_l, b1_l, W1_r, W2_l, b2_l, W2_r,
           trace=False, trace_kwargs=None):
    """Full inputs in, full output out. Shards across 8 NeuronCores."""
    global last_result
    from concourse.bass_utils import run_bass_kernel_spmd

    x = np.asarray(x)
    edge_index = np.asarray(edge_index)
    n_nodes = x.shape[0]
    assert n_nodes % NCORES == 0

    prep, in_maps = make_in_maps(x, edge_index, W1_l, b1_l, W1_r,
                                 W2_l, b2_l, W2_r, n_nodes, NCORES)
    key = (n_nodes,) + tuple(
        (lp["ncols"],) + tuple(blk["nb"] for blk in lp["blocks"])
        for lp in prep["layers"])
    if key not in _cache:
        _cache[key] = build_kernel(n_nodes, NCORES, prep)
    nc = _cache[key]

    res = run_bass_kernel_spmd(nc, in_maps, list(range(NCORES)),
                               trace=trace, **(trace_kwargs or {}))
    last_result = res
    out = np.concatenate([res.results[c]["out"] for c in range(NCORES)],
                         axis=0)
    return out.astype(np.float32)


# revision 33
# speedup vs baseline: 1.2495x; 1.1049x over previous
"""Trainium2 Bass kernel: 2-layer GraphSAGE (mean aggregation), 8-core SPMD.

nn_BiGNN: out = sage2(relu(sage1(x)));  sage(x) = mean_{j->i}(x_j) @ W_l + b_l + x @ W_r
N=50000 nodes, E=800000 edges, d=128, f32 inputs / f32 output.

Strategy (one NeuronCore owns 6250 destination nodes):
  - host: partition edges by destination block, sort by dst, pad per
    128-dst subwindow, split into lo/hi phases (int16 SWDGE index limit),
    equalize batch counts across cores (SPMD). Separate index streams per
    layer: layer 2 gathers from a window-concat AllGather layout.
  - device: SWDGE dma_gather of bf16 source rows round-robined over all 4
    SWDGE queues (4 Q7 core pairs generate descriptors in parallel);
    fused one-hot segment matrices on DVE; TensorE matmul msg^T @ seg
    accumulated per 512-node PSUM window = transposed mean-aggregation;
    1/deg folded into the PSUM evacuation; window tails (weight matmuls,
    relu, transposes, stores) software-pipelined one window behind the
    gather stream; per-window chunked AllGather of h overlaps layer-1
    compute; final layer emits row-major output directly.
"""

import os
import sys
import types

for _p in ("/opt/trn_rl_repo", "/root/.axon_site/_ro/trn_rl_repo",
           "/root/.axon_site"):
    if os.path.isdir(_p) and _p not in sys.path:
        sys.path.insert(0, _p)


def _install_ntff_hook():
    """Provide antenv.axon_hooks (missing in this image) so trace=True can
    capture NTFF profiles through libaxon_pjrt.so."""
    if "antenv.axon_hooks" in sys.modules:
        return
    store = [None]
    mod = types.ModuleType("antenv.axon_hooks")
    mod.set_axon_ntff_profile_hook = lambda h: store.__setitem__(0, h)
    mod.get_axon_ntff_profile_hook = lambda: store[0]
    sys.modules["antenv.axon_hooks"] = mod
    try:
        import antenv
        antenv.axon_hooks = mod
        from trn_agent_boot.trn_boot import _ntff_profile_via_ctypes
        so = "/opt/axon/libaxon_pjrt.so"
        if os.path.exists(so):
            mod.set_axon_ntff_profile_hook(_ntff_profile_via_ctypes(so))
    except Exception:
        pass


_install_ntff_hook()


import numpy as np
import ml_dtypes

import concourse.bass as bass
import concourse.bacc as bacc
import concourse.mybir as mybir
import concourse.tile as tile
from concourse.library_config import mlp as mlp_library

P = 128
D = 128
GMAX = 8  # max batches (1024 idxs) per dma_gather instruction (ucode limit)
NQUEUES = 4  # SWDGE queues; queue k runs on Q7 core pair (2k, 2k+1)
HALF = 32768  # int16 index limit for dma_gather
F32 = mybir.dt.float32
BF16 = mybir.dt.bfloat16
I16 = mybir.dt.int16


def wrap_idx16(arr):
    """[n] int array -> [128, n//16] int16 SWDGE layout (16-partition wrap,
    replicated for the 8 Q7 cores)."""
    n = arr.shape[0]
    assert n % 16 == 0
    w = np.asarray(arr, dtype=np.int16).reshape(n // 16, 16).T  # [16, n/16]
    return np.tile(w, (8, 1))  # [128, n/16]


# ----------------------------------------------------------------- host prep
# AllGather chunking for h (local-row ranges; each chunk is one Shared
# tensor written by one collective). Rebased per-chunk indices must fit
# int16: ncores * chunk_len - 1 <= 32767.
CHUNK_R = [0, 4096, 5632, 6250]  # chunk k covers local rows [R[k], R[k+1])


def _build_stream(edges, ncores, nsub, nwin, spw, nph):
    """Equalize per-(sub, phase) batch counts across cores; build the
    blocks/idx16/slots stream (window-major, phases in order)."""
    nb = np.zeros((nsub, nph), dtype=np.int64)
    for c in range(ncores):
        for t in range(nsub):
            for ph in range(nph):
                n = len(edges[c][nph * t + ph][0])
                nb[t, ph] = max(nb[t, ph], (n + P - 1) // P)
    nb[:, 0] = np.maximum(nb[:, 0], 1)  # ensure each sub has >=1 batch

    blocks = []
    ncols = 0
    for w in range(nwin):
        subs = range(w * spw, min((w + 1) * spw, nsub))
        for ph in range(nph):
            bl = [(t, int(nb[t, ph])) for t in subs]
            nbl = sum(x[1] for x in bl)
            blocks.append(dict(w=w, ph=ph, col0=ncols, nb=nbl, subs=bl))
            ncols += nbl

    idx16 = np.zeros((ncores, P, ncols * 8), dtype=np.int16)
    slots = np.full((ncores, P, ncols), -1.0, dtype=ml_dtypes.bfloat16)
    for c in range(ncores):
        for blk in blocks:
            if blk["nb"] == 0:
                continue
            col = blk["col0"]
            flat_idx = []
            for t, nbt in blk["subs"]:
                s, d = edges[c][nph * t + blk["ph"]]
                n = len(s)
                npad = nbt * P
                si = np.zeros(npad, dtype=np.int64)
                si[:n] = s
                sl = np.full(npad, -1.0, dtype=np.float32)
                sl[:n] = (d % P).astype(np.float32)
                for b in range(nbt):
                    slots[c, :, col + b] = sl[b * P:(b + 1) * P].astype(
                        ml_dtypes.bfloat16)
                flat_idx.append(si)
                col += nbt
            fi = np.concatenate(flat_idx)
            idx16[c, :, blk["col0"] * 8:(blk["col0"] + blk["nb"]) * 8] = \
                wrap_idx16(fi)
    return dict(ncols=ncols, blocks=blocks, idx16=idx16, slots=slots, nph=nph)


def host_prep(edge_index, n_nodes, ncores, win=512):
    """Partition/sort/pad edges; build per-layer phased streams.

    Layer 0 gathers from x in natural node order (2 phases at the int16
    boundary). Layer 1 gathers from the chunked AllGather layout: global
    node (c, r) in chunk k lives at row c*len_k + (r - R[k]) of chunk
    tensor k (3 phases, one per chunk tensor)."""
    npc = n_nodes // ncores
    nsub = (npc + P - 1) // P
    nwin = (npc + win - 1) // win
    spw = win // P
    src_a = np.asarray(edge_index[0], dtype=np.int64)
    dst_a = np.asarray(edge_index[1], dtype=np.int64)

    c_of = src_a // npc
    r_of = src_a % npc
    nchunk = len(CHUNK_R) - 1
    chunk_of = np.searchsorted(CHUNK_R, r_of, side="right") - 1
    clen = np.diff(CHUNK_R)
    # rebased row within the chunk tensor
    rrow_a = c_of * clen[chunk_of] + (r_of - np.asarray(CHUNK_R)[chunk_of])

    invcnt = np.zeros((ncores, 1, npc), dtype=np.float32)
    per_core = []
    for c in range(ncores):
        lo_n = c * npc
        m = (dst_a >= lo_n) & (dst_a < lo_n + npc)
        d = dst_a[m] - lo_n
        invcnt[c, 0] = 1.0 / np.maximum(np.bincount(d, minlength=npc), 1.0)
        order = np.argsort(d, kind="stable")
        per_core.append((m, order, d[order]))

    # per-edge (phase, rebased idx) per layer
    ph0 = (src_a >= HALF).astype(np.int64)
    id0 = src_a - ph0 * HALF
    ph1 = chunk_of
    id1 = rrow_a

    layers = []
    for ph_a, id_a, nph in ((ph0, id0, 2), (ph1, id1, nchunk)):
        edges = [[None] * (nph * nsub) for _ in range(ncores)]
        for c in range(ncores):
            m, order, d_sorted = per_core[c]
            pe = ph_a[m][order]
            ie = id_a[m][order]
            sub = d_sorted // P
            for t in range(nsub):
                ms = sub == t
                for ph in range(nph):
                    mp = ms & (pe == ph)
                    edges[c][nph * t + ph] = (ie[mp], d_sorted[mp])
        layers.append(_build_stream(edges, ncores, nsub, nwin, spw, nph))

    return dict(npc=npc, nsub=nsub, nwin=nwin, win=win, invcnt=invcnt,
                layers=layers)


# -------------------------------------------------------------- kernel build
def build_kernel(n_nodes, ncores, prep, nb_onehot=8):
    npc, nwin, win = prep["npc"], prep["nwin"], prep["win"]
    L0, L1 = prep["layers"]
    spw = win // P

    nc = bacc.Bacc(None, num_swdge_queues=NQUEUES)

    xtab = nc.declare_dram_parameter("xtab", [n_nodes, D], BF16, isOutput=False)
    xT_d = nc.declare_dram_parameter("xT", [D, npc], F32, isOutput=False)
    idx0_d = nc.declare_dram_parameter("idx0", [P, L0["ncols"] * 8], I16, isOutput=False)
    slots0_d = nc.declare_dram_parameter("slots0", [P, L0["ncols"]], BF16, isOutput=False)
    idx1_d = nc.declare_dram_parameter("idx1", [P, L1["ncols"] * 8], I16, isOutput=False)
    slots1_d = nc.declare_dram_parameter("slots1", [P, L1["ncols"]], BF16, isOutput=False)
    invcnt_d = nc.declare_dram_parameter("invcnt", [P, npc], F32, isOutput=False)
    W1l_d = nc.declare_dram_parameter("W1l", [D, D], F32, isOutput=False)
    W1r_d = nc.declare_dram_parameter("W1r", [D, D], F32, isOutput=False)
    W2l_d = nc.declare_dram_parameter("W2l", [D, D], F32, isOutput=False)
    W2r_d = nc.declare_dram_parameter("W2r", [D, D], F32, isOutput=False)
    b1_d = nc.declare_dram_parameter("b1", [D, 1], F32, isOutput=False)
    b2row_d = nc.declare_dram_parameter("b2row", [P, D], F32, isOutput=False)
    iota_d = nc.declare_dram_parameter("iota", [P, P], BF16, isOutput=False)
    ident_d = nc.declare_dram_parameter("ident", [P, P], F32, isOutput=False)
    out_d = nc.declare_dram_parameter("out", [npc, D], F32, isOutput=True)

    from contextlib import ExitStack
    with tile.TileContext(nc) as tc, ExitStack() as es:
        dram = es.enter_context(tc.tile_pool(name="dram", bufs=1, space="DRAM"))
        h_local = dram.tile([npc, D], BF16, tag="hloc")
        # AllGather chunk tensors (each written by exactly one collective)
        h_chunks = []
        for k in range(len(CHUNK_R) - 1):
            clen = CHUNK_R[k + 1] - CHUNK_R[k]
            hck = dram.tile([ncores * clen, D], BF16, tag=f"hck{k}",
                            name=f"hck{k}", addr_space="Shared")
            h_chunks.append(hck)

        const = es.enter_context(tc.tile_pool(name="const", bufs=1))
        sb = es.enter_context(tc.tile_pool(name="sb", bufs=1))
        msgp = es.enter_context(tc.tile_pool(name="msgp", bufs=16))
        segp = es.enter_context(tc.tile_pool(name="segp", bufs=10))
        aggp = es.enter_context(tc.tile_pool(name="aggp", bufs=3))
        rowp = es.enter_context(tc.tile_pool(name="rowp", bufs=4))
        psA = es.enter_context(tc.tile_pool(name="psA", bufs=3, space="PSUM"))
        psB = es.enter_context(tc.tile_pool(name="psB", bufs=2, space="PSUM"))
        psT = es.enter_context(tc.tile_pool(name="psT", bufs=3, space="PSUM"))

        nc.gpsimd.load_library(mlp_library)

        idx0_sb = const.tile([P, L0["ncols"] * 8], I16, tag="idx0")
        slots0_sb = const.tile([P, L0["ncols"]], BF16, tag="slots0")
        idx1_sb = const.tile([P, L1["ncols"] * 8], I16, tag="idx1")
        slots1_sb = const.tile([P, L1["ncols"]], BF16, tag="slots1")
        invcnt_sb = const.tile([P, npc], F32, tag="invcnt")
        iota_sb = const.tile([P, P], BF16, tag="iota")
        ident_sb = const.tile([P, P], F32, tag="ident")
        W1l_sb = const.tile([D, D], F32, tag="W1l")
        W1r_sb = const.tile([D, D], F32, tag="W1r")
        W2l_sb = const.tile([D, D], F32, tag="W2l")
        W2r_sb = const.tile([D, D], F32, tag="W2r")
        b1_sb = const.tile([D, 1], F32, tag="b1")
        b2row_sb = const.tile([P, D], F32, tag="b2row")
        xT_sb = sb.tile([D, npc], F32, tag="xT")
        hT_sb = sb.tile([D, npc], F32, tag="hT")

        # gather-critical loads first on the Sync HWDGE queue; the rest on
        # the Scalar HWDGE queue so the two rings drain in parallel
        for t, dd in [(idx0_sb, idx0_d), (slots0_sb, slots0_d),
                      (iota_sb, iota_d), (W1l_sb, W1l_d), (W1r_sb, W1r_d),
                      (b1_sb, b1_d), (xT_sb, xT_d)]:
            nc.sync.dma_start(out=t[:], in_=dd[:])
        for t, dd in [(idx1_sb, idx1_d), (slots1_sb, slots1_d),
                      (invcnt_sb, invcnt_d), (ident_sb, ident_d),
                      (W2l_sb, W2l_d), (W2r_sb, W2r_d), (b2row_sb, b2row_d)]:
            nc.scalar.dma_start(out=t[:], in_=dd[:])

        gq = [0]  # round-robin SWDGE queue counter (4 Q7 core pairs)

        def emit_cc(k):
            r0, r1 = CHUNK_R[k], CHUNK_R[k + 1]
            nc.gpsimd.collective_compute(
                "AllGather", mybir.AluOpType.bypass,
                replica_groups=[list(range(ncores))],
                ins=[h_local[r0:r1, :]],
                outs=[h_chunks[k][:]])

        # emit chunk k's collective after tail(last_w + 1): by then the
        # chunk's stores have landed (early in window last_w+1's gathers),
        # so the Pool sequencer never blocks on the collective's wait
        cc_after_tail = {}
        for k in range(len(CHUNK_R) - 1):
            last_w = (CHUNK_R[k + 1] - 1) // win
            cc_after_tail[last_w + 1] = k
        cc_at_end = [k for w, k in cc_after_tail.items() if w >= nwin]
        cc_after_tail = {w: k for w, k in cc_after_tail.items() if w < nwin}

        def emit_layer(layer, lp, tabs, idx_sb, slots_sb, chunked_cc,
                       cc_pending=()):
            cc_pending = set(cc_pending)
            ncols, blocks = lp["ncols"], lp["blocks"]
            ngrp = (ncols + nb_onehot - 1) // nb_onehot
            segs = []
            for g in range(ngrp):
                nbg = min(nb_onehot, ncols - g * nb_onehot)
                seg = segp.tile([P, nb_onehot, P], BF16, tag="seg",
                                name=f"seg{layer}_{g}")
                g0 = g * nb_onehot
                nc.vector.tensor_tensor(
                    out=seg[:, :nbg, :],
                    in0=iota_sb[:, None, :].to_broadcast([P, nbg, P]),
                    in1=slots_sb[:, g0:g0 + nbg, None].to_broadcast([P, nbg, P]),
                    op=mybir.AluOpType.is_equal,
                )
                segs.append(seg)

            def tail(w, agg_ps):
                n0 = w * win
                wn = min(win, npc - n0)
                nsw = (wn + P - 1) // P
                aggTs = aggp.tile([P, win], F32, tag="aggTs",
                                  name=f"aggTs{layer}_{w}")
                nc.vector.tensor_tensor(
                    out=aggTs[:, :wn], in0=agg_ps[:, :wn],
                    in1=invcnt_sb[:, n0:n0 + wn], op=mybir.AluOpType.mult)

                if layer == 0:
                    ab_ps = psB.tile([P, win], F32, tag="AB", name=f"ab{w}")
                    nc.tensor.matmul(out=ab_ps[:, :wn], lhsT=W1l_sb[:],
                                     rhs=aggTs[:, :wn], start=True, stop=False)
                    nc.tensor.matmul(out=ab_ps[:, :wn], lhsT=W1r_sb[:],
                                     rhs=xT_sb[:, n0:n0 + wn], start=False,
                                     stop=True)
                    nc.scalar.activation(
                        out=hT_sb[:, n0:n0 + wn], in_=ab_ps[:, :wn],
                        func=mybir.ActivationFunctionType.Relu,
                        bias=b1_sb[:, 0:1], scale=1.0)
                    for j in range(nsw):
                        r0 = n0 + j * P
                        ns = min(P, npc - r0)
                        tr_ps = psT.tile([P, P], F32, tag="tr",
                                         name=f"tr{w}_{j}")
                        nc.tensor.transpose(out=tr_ps[:ns, :],
                                            in_=hT_sb[:, r0:r0 + ns],
                                            identity=ident_sb[:])
                        hrow = rowp.tile([P, D], BF16, tag="hrow",
                                         name=f"hrow{w}_{j}")
                        nc.scalar.activation(
                            out=hrow[:ns, :], in_=tr_ps[:ns, :],
                            func=mybir.ActivationFunctionType.Copy, scale=1.0)
                        nc.sync.dma_start(out=h_local[r0:r0 + ns, :],
                                          in_=hrow[:ns, :])
                else:
                    for j in range(nsw):
                        r0 = n0 + j * P
                        ns = min(P, npc - r0)
                        o_ps = psT.tile([P, P], F32, tag="tr",
                                        name=f"ops{w}_{j}")
                        nc.tensor.matmul(out=o_ps[:ns, :],
                                         lhsT=aggTs[:, j * P:j * P + ns],
                                         rhs=W2l_sb[:], start=True, stop=False)
                        nc.tensor.matmul(out=o_ps[:ns, :],
                                         lhsT=hT_sb[:, r0:r0 + ns],
                                         rhs=W2r_sb[:], start=False, stop=True)
                        orow = rowp.tile([P, D], F32, tag="orow",
                                         name=f"orow{w}_{j}")
                        nc.vector.tensor_tensor(
                            out=orow[:ns, :], in0=o_ps[:ns, :],
                            in1=b2row_sb[:ns, :], op=mybir.AluOpType.add)
                        nc.sync.dma_start(out=out_d[r0:r0 + ns, :],
                                          in_=orow[:ns, :])

            for w in range(nwin):
                agg_ps = psA.tile([P, win], F32, tag="aggT",
                                  name=f"agg{layer}_{w}")
                sub_of_b = {}
                for blk in blocks:
                    if blk["w"] != w or blk["nb"] == 0:
                        continue
                    col = blk["col0"]
                    for t, nbt in blk["subs"]:
                        for bi in range(nbt):
                            sub_of_b[col + bi] = t
                        col += nbt
                win_first_b = min(sub_of_b)
                win_last_b = max(sub_of_b)

                for blk in blocks:
                    if blk["w"] != w or blk["nb"] == 0:
                        continue
                    if blk["ph"] in cc_pending:
                        # deferred final AllGather chunk: emitted just before
                        # the first gather that reads it, so earlier-phase
                        # gathers dispatch without waiting on layer-1 stores
                        emit_cc(blk["ph"])
                        cc_pending.discard(blk["ph"])
                    tab = tabs[blk["ph"]]
                    for c0 in range(0, blk["nb"], GMAX):
                        cn = min(GMAX, blk["nb"] - c0)
                        msg = msgp.tile([P, GMAX, D], BF16, tag="msg",
                                        name=f"msg{layer}_{w}_{blk['ph']}_{c0}")
                        nidx = cn * P
                        b0 = blk["col0"] + c0
                        nc.gpsimd.dma_gather(
                            out_ap=msg[:, :cn, :],
                            in_ap=tab,
                            idxs_ap=idx_sb[:, b0 * 8:(b0 + cn) * 8],
                            num_idxs=nidx,
                            num_idxs_reg=nidx,
                            elem_size=D,
                            queue_num=gq[0] % NQUEUES,
                        )
                        gq[0] += 1
                        for bi in range(cn):
                            b = b0 + bi
                            t = sub_of_b[b]
                            j = t - w * spw
                            nsl = min(P, npc - t * P)
                            nc.tensor.matmul(
                                out=agg_ps[:, j * P:j * P + nsl],
                                lhsT=msg[:, bi, :],
                                rhs=segs[b // nb_onehot][:, b % nb_onehot, :nsl],
                                start=(b == win_first_b),
                                stop=(b == win_last_b),
                            )

                tail(w, agg_ps)
                if chunked_cc and w in cc_after_tail:
                    emit_cc(cc_after_tail[w])
            for k in sorted(cc_pending):
                emit_cc(k)

        emit_layer(0, L0, [xtab[0:HALF, :], xtab[HALF:n_nodes, :]],
                   idx0_sb, slots0_sb, chunked_cc=True)
        emit_layer(1, L1, [t[:] for t in h_chunks],
                   idx1_sb, slots1_sb, chunked_cc=False,
                   cc_pending=cc_at_end)

    nc.finalize()
    return nc


# ---------------------------------------------------------------- in_maps
def make_in_maps(x, edge_index, W1_l, b1_l, W1_r, W2_l, b2_l, W2_r,
                 n_nodes, ncores, win=512):
    prep = host_prep(edge_index, n_nodes, ncores, win=win)
    npc = prep["npc"]
    L0, L1 = prep["layers"]
    x = np.asarray(x, dtype=np.float32)
    xtab = x.astype(ml_dtypes.bfloat16)
    xT = np.ascontiguousarray(x.T)
    iota = np.tile(np.arange(P, dtype=np.float32)[None, :], (P, 1)).astype(
        ml_dtypes.bfloat16)
    ident = np.eye(P, dtype=np.float32)
    common = dict(
        xtab=xtab,
        W1l=np.asarray(W1_l, np.float32), W1r=np.asarray(W1_r, np.float32),
        W2l=np.asarray(W2_l, np.float32), W2r=np.asarray(W2_r, np.float32),
        b1=np.asarray(b1_l, np.float32).reshape(D, 1),
        b2row=np.tile(np.asarray(b2_l, np.float32).reshape(1, D), (P, 1)),
        iota=iota, ident=ident,
    )
    in_maps = []
    for c in range(ncores):
        in_maps.append(dict(
            common,
            xT=np.ascontiguousarray(xT[:, c * npc:(c + 1) * npc]),
            idx0=L0["idx16"][c], slots0=L0["slots"][c],
            idx1=L1["idx16"][c], slots1=L1["slots"][c],
            invcnt=np.tile(prep["invcnt"][c], (P, 1)),
        ))
    return prep, in_maps


# ------------------------------------------------------------------ kernel()
N_NODES = 50000
NCORES = 8

_cache = {}
last_result = None  # BassKernelResults of the most recent run (for test.py)


def kernel(x, edge_index, W1_l, b1_l, W1_r, W2_l, b2_l, W2_r,
           trace=False, trace_kwargs=None):
    """Full inputs in, full output out. Shards across 8 NeuronCores."""
    global last_result
    from concourse.bass_utils import run_bass_kernel_spmd

    x = np.asarray(x)
    edge_index = np.asarray(edge_index)
    n_nodes = x.shape[0]
    assert n_nodes % NCORES == 0

    prep, in_maps = make_in_maps(x, edge_index, W1_l, b1_l, W1_r,
                                 W2_l, b2_l, W2_r, n_nodes, NCORES)
    key = (n_nodes,) + tuple(
        (lp["ncols"],) + tuple(blk["nb"] for blk in lp["blocks"])
        for lp in prep["layers"])
    if key not in _cache:
        _cache[key] = build_kernel(n_nodes, NCORES, prep)
    nc = _cache[key]

    res = run_bass_kernel_spmd(nc, in_maps, list(range(NCORES)),
                               trace=trace, **(trace_kwargs or {}))
    last_result = res
    out = np.concatenate([res.results[c]["out"] for c in range(NCORES)],
                         axis=0)
    return out.astype(np.float32)
